# revision 51
# baseline (speedup 1.0000x reference)
"""Cross-attention Trainium2 kernel (nn_CrossAttention_7627861918199).

Full-input contract: kernel(**inputs) takes the unsharded numpy inputs and
returns the full [B, NQ, D] float32 output.

Sharding: 8 cores = (batch b, head-group hg); core c handles batch c//2 and
heads [4*(c%2), 4*(c%2)+4) for ALL nq=1024 queries.  Tensor-parallel over
heads: Wq/Wkv are split column-wise (256 inner dims per core), Wo row-wise;
each core emits a partial y = O_hg @ Wo_hg^T and the host sums the two
partials per batch during unshard (the "all-reduce after to_out").  This
halves the K/V projection work vs. query-sharding (context is projected
once per head-group, not once per query-half).

Per-core pipeline (all matmuls bf16, fp32 accumulate):
  qT = Wq_hg @ xT          [256, 1024]
  kT = Wk_hg @ ctxT        [256, 4096]
  v  = ctx @ Wv_hg^T       [4096, 4x65]  (65th col = ones for softmax sums)
  4 segments (head-pair hp x query-half qh), 32 kv-chunks each:
    S^T chunk [128kv, 1024] = k @ qT  -> exp (ACT, scale=1/8) -> P^T bf16
    -> av[65, 512] += v_aug^T @ P^T   (PSUM accum over 32 chunks)
    segment end: stage av PSUM->SBUF (frees banks), fast-reciprocal of the
    two sum rows in place, DMA-broadcast, normalize, write O^T
  y_partial = O^T.T @ Wo_hg^T            (bo added on host)
Input DMAs are spread across 5 engine queues; projections are interleaved
into segments with PE slack (V+Kic0 in seg0, Kic1 split over seg1/seg2,
y projection for the first query-half inside seg3).
"""

import numpy as np
import ml_dtypes

HEADS = 8
DIM_HEAD = 64
D = 512          # QUERY_DIM == full inner dim
B, NQ, NKV = 4, 1024, 4096
N_CORES = 8
NHL = 4          # heads per core
INNER = NHL * DIM_HEAD       # 256 local inner dims
P = 128
KC = D // P                  # 4 contraction chunks of 128 (over QUERY_DIM)
ICK = INNER // P             # 2 local inner chunks of 128
NCHUNK = NKV // P            # 32 kv chunks of 128
NT = NKV // 512              # 8 n-tiles for kT
NQH = NQ // 512              # 2 query halves
BF16 = ml_dtypes.bfloat16

_PROGRAMS = {}


def _build(need_mask: bool, num_devices: int = N_CORES):
    import concourse.mybir as mybir
    import concourse.tile as tile
    from concourse import bacc

    dt = mybir.dt
    f32, bf = dt.float32, dt.bfloat16

    nc = bacc.Bacc("TRN2", target_bir_lowering=False, debug=False,
                   num_devices=num_devices)

    # all inputs host-swizzled to per-partition-contiguous layouts so each
    # DMA is 128 large contiguous runs (fast descriptor generation)
    xT = nc.dram_tensor("xT", [P, KC, NQ], bf, kind="ExternalInput").ap()
    ctxT = nc.dram_tensor("ctxT", [NT, P, KC, 512], bf,
                          kind="ExternalInput").ap()
    wqT = nc.dram_tensor("wqT", [P, KC, INNER], bf, kind="ExternalInput").ap()
    wkT = nc.dram_tensor("wkT", [P, KC, INNER], bf, kind="ExternalInput").ap()
    wvT = nc.dram_tensor("wvT", [P, KC, INNER], bf, kind="ExternalInput").ap()
    woT = nc.dram_tensor("woT", [P, ICK, D], bf, kind="ExternalInput").ap()
    if need_mask:
        maskb = nc.dram_tensor("maskb", [P, NCHUNK], f32,
                               kind="ExternalInput").ap()
    y = nc.dram_tensor("y", [NQ, D], f32, kind="ExternalOutput").ap()

    Exp = mybir.ActivationFunctionType.Exp

    with tile.TileContext(nc) as tc:
        with tc.tile_pool(name="big", bufs=1) as big, \
             tc.tile_pool(name="work", bufs=3) as work, \
             tc.tile_pool(name="pTp", bufs=10) as pTp, \
             tc.tile_pool(name="dram", bufs=2, space="DRAM") as dram, \
             tc.tile_pool(name="proj_ps", bufs=2, space="PSUM") as proj_ps, \
             tc.tile_pool(name="score_ps", bufs=2, space="PSUM") as score_ps, \
             tc.tile_pool(name="av_ps", bufs=2, space="PSUM") as av_ps:

            ctx_sb = big.tile([P, NT, KC, 512], bf, name="ctx_sb")
            x_sb = big.tile([P, KC, NQ], bf, name="x_sb")
            wq_sb = big.tile([P, KC, INNER], bf, name="wq_sb")
            wk_sb = big.tile([P, KC, INNER], bf, name="wk_sb")
            wv_sb = big.tile([P, KC, INNER], bf, name="wv_sb")
            wo_sb = big.tile([P, ICK, D], bf, name="wo_sb")
            q_sb = big.tile([P, ICK, NQ], bf, name="q_sb")
            k_sb = big.tile([P, ICK, NKV], bf, name="k_sb")
            v_sb = big.tile([P, NCHUNK, NHL, DIM_HEAD + 1], bf, name="v_sb")
            o_sb = big.tile([P, ICK, NQ], bf, name="o_sb")
            if need_mask:
                mb_sb = big.tile([P, NCHUNK], f32, name="mb_sb")

            # ---- input DMAs spread over the 3 DMA-capable queues ----
            # priority: the critical chain is x+wq (Q proj) and wk+nt0
            # (first K) -> first scores -> first EXP.  ctx nt4-7 are gated
            # behind a tiny DVE memset so their bytes don't compete with
            # the critical inputs for DMA bandwidth (issue order alone is
            # not enough: the DMA queues all drain concurrently).
            nc.sync.dma_start(x_sb[:], xT)
            nc.scalar.dma_start(wk_sb[:], wkT)
            nc.scalar.dma_start(wq_sb[:], wqT)
            nc.scalar.dma_start(wv_sb[:], wvT)
            for nt in range(4):
                nc.gpsimd.dma_start(ctx_sb[:, nt, :, :], ctxT[nt])
            nc.scalar.dma_start(wo_sb[:], woT)
            if need_mask:
                nc.scalar.dma_start(mb_sb[:], maskb[:])
            nc.vector.memset(v_sb[:, :, :, DIM_HEAD], 1.0)

            def emit_q():
                # Q proj: qT[256, 1024]
                for ic in range(ICK):
                    for qh in range(NQH):
                        ps = proj_ps.tile([P, 512], f32, name="ps_proj",
                                          tag="proj")
                        for kc in range(KC):
                            nc.tensor.matmul(
                                ps, wq_sb[:, kc, ic * P:(ic + 1) * P],
                                x_sb[:, kc, qh * 512:(qh + 1) * 512],
                                start=(kc == 0), stop=(kc == KC - 1))
                        nc.vector.tensor_copy(
                            out=q_sb[:, ic, qh * 512:(qh + 1) * 512], in_=ps)

            def emit_k(ic, nt):
                ps = proj_ps.tile([P, 512], f32, name="ps_proj", tag="proj")
                for kc in range(KC):
                    nc.tensor.matmul(
                        ps, wk_sb[:, kc, ic * P:(ic + 1) * P],
                        ctx_sb[:, nt, kc, :],
                        start=(kc == 0), stop=(kc == KC - 1))
                nc.vector.tensor_copy(
                    out=k_sb[:, ic, nt * 512:(nt + 1) * 512], in_=ps)

            def emit_v(j):
                # v rows for kv chunk j, all 4 local heads: [128kv, 256]
                nt, jo = j // 4, (j % 4) * P
                ps = proj_ps.tile([P, 512], f32, name="ps_proj", tag="proj")
                for kc in range(KC):
                    nc.tensor.matmul(
                        ps[:, 0:INNER], ctx_sb[:, nt, kc, jo:jo + P],
                        wv_sb[:, kc, :], start=(kc == 0), stop=(kc == KC - 1))
                nc.vector.tensor_copy(
                    out=v_sb[:, j, :, 0:DIM_HEAD],
                    in_=ps[:, 0:INNER].rearrange("p (h d) -> p h d", h=NHL))

            y_eng = [nc.gpsimd, nc.sync, nc.gpsimd, nc.sync,
                     nc.scalar, nc.sync, nc.gpsimd, nc.scalar]

            def emit_oproj(qc):
                # y_partial chunk = O[qc] @ Wo_hg^T
                ps = proj_ps.tile([P, 512], f32, name="ps_proj", tag="proj")
                for ic in range(ICK):
                    nc.tensor.matmul(
                        ps, o_sb[:, ic, qc * P:(qc + 1) * P],
                        wo_sb[:, ic, :], start=(ic == 0), stop=(ic == ICK - 1))
                y_sb = work.tile([P, D], f32, name="y_sb", tag="y_sb")
                nc.vector.tensor_copy(out=y_sb, in_=ps)
                y_eng[qc].dma_start(
                    y[qc * P:qc * P + DIM_HEAD, :], y_sb[0:DIM_HEAD, :])
                y_eng[(qc + 1) % 8].dma_start(
                    y[qc * P + DIM_HEAD:(qc + 1) * P, :],
                    y_sb[DIM_HEAD:P, :])

            ones64 = big.tile([1, DIM_HEAD], bf, name="ones64")
            nc.vector.memset(ones64[:], 1.0)

            def emit_normalize(av0, av1, hp, qh, last=False):
                # The z chain (pack -> recip -> unpack/DRAM -> broadcast)
                # packs the 1024 z values across 128 partitions so the
                # reciprocal is partition-parallel (free size 8).
                avs = work.tile([DIM_HEAD + 1, 2, 512], f32, name="avs",
                                tag="avs")
                nc.vector.tensor_copy(out=avs[:, 0, :], in_=av0[:])
                nc.vector.tensor_copy(out=avs[:, 1, :], in_=av1[:])
                zp = work.tile([P, 8], f32, name="zp", tag="zp")
                nc.gpsimd.dma_start(
                    zp[:], avs[DIM_HEAD:DIM_HEAD + 1, :, :])
                zr = work.tile([P, 8], f32, name="zr", tag="zr")
                nc.vector.reciprocal(zr[:], zp[:])
                o_tmp = work.tile([DIM_HEAD, 2, 512], bf, name="o_tmp",
                                  tag="o_tmp")
                if last:
                    # PE is drained here and the av-pool banks are free:
                    # unpack 1/z (cast bf16) to one row, then broadcast it
                    # with a rank-1 ones-matmul straight into PSUM (no
                    # DRAM trip; bf16 because fp32 matmuls run at 1/4 rate)
                    zrb = work.tile([P, 8], bf, name="zrb", tag="zrb")
                    nc.vector.tensor_copy(out=zrb[:], in_=zr[:])
                    zrow = work.tile([1, 2, 512], bf, name="zrow",
                                     tag="zrow")
                    nc.gpsimd.dma_start(zrow[:], zrb[:])
                    rbc0 = av_ps.tile([DIM_HEAD, 512], f32, name="rbc0",
                                      tag="av")
                    rbc1 = av_ps.tile([DIM_HEAD, 512], f32, name="rbc1",
                                      tag="av")
                    nc.tensor.matmul(rbc0, ones64[:], zrow[:, 0, :],
                                     start=True, stop=True)
                    nc.tensor.matmul(rbc1, ones64[:], zrow[:, 1, :],
                                     start=True, stop=True)
                    nc.vector.tensor_mul(o_tmp[:, 0, :],
                                         avs[0:DIM_HEAD, 0, :], rbc0[:])
                    nc.vector.tensor_mul(o_tmp[:, 1, :],
                                         avs[0:DIM_HEAD, 1, :], rbc1[:])
                else:
                    rec_dr = dram.tile([1, 2, DIM_HEAD, 8], f32,
                                       name="rec_dr", tag="rec_dr")
                    nc.gpsimd.dma_start(
                        rec_dr.rearrange("x a p f -> (x a p) f"), zr[:])
                    rec_bc = work.tile([DIM_HEAD, 2, 512], f32,
                                       name="rec_bc", tag="rec_bc")
                    nc.gpsimd.dma_start(
                        rec_bc[:],
                        rec_dr.rearrange("x a p f -> x a (p f)")
                        .to_broadcast([DIM_HEAD, 2, 512]))
                    nc.vector.tensor_mul(o_tmp[:, 0, :],
                                         avs[0:DIM_HEAD, 0, :],
                                         rec_bc[:, 0, :])
                    nc.vector.tensor_mul(o_tmp[:, 1, :],
                                         avs[0:DIM_HEAD, 1, :],
                                         rec_bc[:, 1, :])
                nc.sync.dma_start(
                    o_sb[0:DIM_HEAD, hp, qh * 512:(qh + 1) * 512],
                    o_tmp[:, 0, :])
                nc.gpsimd.dma_start(
                    o_sb[DIM_HEAD:P, hp, qh * 512:(qh + 1) * 512],
                    o_tmp[:, 1, :])

            # prologue: K ic0 nt0-3 and v chunks 0,1 run in the DMA shadow
            # (they only need wk/wv + early ctx slices), interleaved with
            # Q proj so segment (0,0) never waits at its head.
            emit_k(0, 0)
            emit_q()
            emit_k(0, 1)
            # release the gate for ctx nt4-7 (DVE reaches this after the
            # K(0,1) eviction, by which time the critical inputs are in)
            nc.vector.memset(ctx_sb[:, 4:8, 0, 0], 0.0)
            for nt in range(4, NT):
                nc.gpsimd.dma_start(ctx_sb[:, nt, :, :], ctxT[nt])
            emit_v(0)
            emit_v(1)
            emit_k(0, 2)
            emit_k(0, 3)

            # AV matmuls lag scores/exp by a variable SKEW, carried ACROSS
            # segment boundaries so the ACT engine never drains; each
            # segment's normalize is emitted when its last AV pops.  The
            # lag is deep during PE-heavy seg0/seg1 (deferring AV work into
            # later ACT-gated segments' slack) and drains back by the tail.
            pend = []

            def pop_pend():
                j, pT, emit_av, fin = pend.pop(0)
                emit_av(j, pT)
                if fin is not None:
                    fin()

            for hp in range(ICK):          # head pair = inner chunk
                h0, h1 = 2 * hp, 2 * hp + 1
                for qh in range(NQH):
                    seg0 = (hp == 0 and qh == 0)
                    seg1 = (hp == 0 and qh == 1)
                    seg2 = (hp == 1 and qh == 0)
                    seg3 = (hp == 1 and qh == 1)
                    av0 = av_ps.tile([DIM_HEAD + 1, 512], f32, name="av0",
                                     tag="av")
                    av1 = av_ps.tile([DIM_HEAD + 1, 512], f32, name="av1",
                                     tag="av")

                    def emit_av(j, pT, av0=av0, av1=av1, h0=h0, h1=h1):
                        nc.tensor.matmul(
                            av0, v_sb[:, j, h0, :], pT[:, 0:512],
                            start=(j == 0), stop=(j == NCHUNK - 1))
                        nc.tensor.matmul(
                            av1, v_sb[:, j, h1, :], pT[:, 512:1024],
                            start=(j == 0), stop=(j == NCHUNK - 1))

                    def fin(av0=av0, av1=av1, hp=hp, qh=qh, last=seg3):
                        emit_normalize(av0, av1, hp, qh, last)

                    for j in range(NCHUNK):
                        if seg0:
                            if j + 2 < NCHUNK:
                                emit_v(j + 2)
                            if j % 4 == 2 and 8 <= j <= 22:
                                emit_k(0, (j + 6) // 4)
                        if seg1 and j in (21, 25):
                            emit_k(1, (j - 21) // 4)
                        if seg2 and j % 4 == 0 and j < 24:
                            emit_k(1, j // 4 + 2)
                        if seg3 and j in (14, 17, 20, 23):
                            emit_oproj((j - 14) // 3)

                        sc = score_ps.tile([P, 1024], f32, name="sc",
                                           tag="sc")
                        nc.tensor.matmul(
                            sc[:, 0:512],
                            k_sb[0:DIM_HEAD, hp, j * P:(j + 1) * P],
                            q_sb[0:DIM_HEAD, hp, qh * 512:(qh + 1) * 512],
                            start=True, stop=True)
                        nc.tensor.matmul(
                            sc[:, 512:1024],
                            k_sb[DIM_HEAD:P, hp, j * P:(j + 1) * P],
                            q_sb[DIM_HEAD:P, hp, qh * 512:(qh + 1) * 512],
                            start=True, stop=True)
                        pT = pTp.tile([P, 1024], bf, name="pT", tag="pT")
                        if need_mask:
                            nc.scalar.activation(
                                pT[:], sc[:], Exp,
                                bias=mb_sb[:, j, None], scale=0.125)
                        else:
                            nc.scalar.activation(pT[:], sc[:], Exp,
                                                 scale=0.125)
                        pend.append(
                            (j, pT, emit_av, fin if j == NCHUNK - 1 else None))
                        if seg0 or seg1:
                            skew = 8
                        elif seg2:
                            skew = max(3, 8 - j // 5)
                        else:
                            skew = 3
                        while len(pend) > skew:
                            pop_pend()
                        if seg3 and j >= NCHUNK - 3 and pend:
                            pop_pend()     # fast drain at the very end
            while pend:
                pop_pend()

            # ---- y_partial qc 4-7 (qc 0-3 emitted inside seg3) ----
            for qc in range(4, NQ // P):
                emit_oproj(qc)

    nc.compile()
    return nc


def _get_program(need_mask: bool):
    if need_mask not in _PROGRAMS:
        _PROGRAMS[need_mask] = _build(need_mask)
    return _PROGRAMS[need_mask]


def _prep_inputs(x, context, mask, Wq, Wkv, Wo, bo):
    """Host-side shard + transpose + cast. Returns (in_maps, need_mask, bo)."""
    x = np.asarray(x, dtype=np.float32)
    context = np.asarray(context, dtype=np.float32)
    mask = np.asarray(mask)
    Wq = np.asarray(Wq, dtype=np.float32)
    Wkv = np.asarray(Wkv, dtype=np.float32)
    Wo = np.asarray(Wo, dtype=np.float32)
    bo = np.asarray(bo, dtype=np.float32)

    need_mask = not bool(mask.all())

    def swz(aT, ck):
        # [ck*128, N] -> per-partition-contiguous [128, ck, N]
        return np.ascontiguousarray(
            aT.reshape(ck, P, -1).transpose(1, 0, 2)).astype(BF16)

    xTs = [swz(x[b].T, KC) for b in range(B)]
    # ctx: [512, 4096] -> [nt, p, kc, 512]
    ctxTs = [np.ascontiguousarray(
        context[b].T.reshape(KC, P, NT, 512).transpose(2, 1, 0, 3))
        .astype(BF16) for b in range(B)]
    wqTs, wkTs, wvTs, woTs = [], [], [], []
    for hg in range(2):
        sl = slice(hg * INNER, (hg + 1) * INNER)
        wqTs.append(swz(Wq[sl].T, KC))
        wkTs.append(swz(Wkv[:D][sl].T, KC))
        wvTs.append(swz(Wkv[D:][sl].T, KC))
        woTs.append(swz(Wo[:, sl].T, ICK))
    if need_mask:
        # additive pre-exp bias: 0 where visible, -1e30 where masked
        mb = [np.where(mask[b], 0.0, -1e30).astype(np.float32)
              .reshape(NCHUNK, P).T.copy() for b in range(B)]

    in_maps = []
    for c in range(N_CORES):
        b, hg = divmod(c, 2)
        m = {
            "xT": xTs[b], "ctxT": ctxTs[b],
            "wqT": wqTs[hg], "wkT": wkTs[hg], "wvT": wvTs[hg],
            "woT": woTs[hg],
        }
        if need_mask:
            m["maskb"] = mb[b]
        in_maps.append(m)
    return in_maps, need_mask, bo


def run_sharded(inputs, trace=False):
    """Run on 8 cores; returns (full_output, BassKernelResults)."""
    from concourse import bass_utils
    in_maps, need_mask, bo = _prep_inputs(**inputs)
    nc = _get_program(need_mask)
    res = bass_utils.run_bass_kernel_spmd(
        nc, in_maps, core_ids=list(range(N_CORES)), trace=trace)
    out = np.empty((B, NQ, D), dtype=np.float32)
    for b in range(B):
        out[b] = res.results[2 * b]["y"]
        out[b] += res.results[2 * b + 1]["y"]
        out[b] += bo
    return out, res


def kernel(**inputs) -> np.ndarray:
    out, _ = run_sharded(inputs, trace=False)
    return out
